# revision 3
# baseline (speedup 1.0000x reference)
# Trainium2 Bass kernel for nn_DiversityLoss (segment_reduce).
#
# reference:
#   sums   = segment_sum(embeddings, labels, C)        # [C, D]
#   counts = segment_sum(ones, labels, C)              # [C]
#   return -mean(var(sums / counts, axis=0, ddof=1))
#
# Strategy (data-parallel across 8 NeuronCores):
#   - Shard N=1M rows into 8 shards of 125k rows.
#   - On each core, compute the per-class partial sums of its shard with a
#     one-hot matmul on the Tensor engine:
#        for each 128-row tile t:
#           onehot[p, c] = (label[row p of t] == c)      (DVE is_equal vs iota)
#           psum[D, C]  += emb_tile[K=128rows, M=128D]^T @ onehot[K=128, N=C]
#     accumulated in PSUM (fp32) across all 980 tiles, then flushed to DRAM.
#   - Host: sum the 8 partial [D, C] outputs, counts via bincount on the
#     labels (0.2% of input bytes), then means/variance in float64.
#
# Layout prep on host (pure layout/dtype glue, no reduction math):
#   - embeddings cast fp32->fp16 and laid out [p, t, d]-contiguous per core so
#     each SBUF partition's DMA stream is fully contiguous.
#   - labels as fp16 in [p, t] layout (values 0..999 are exact in fp16).

import numpy as np

N = 1_000_000
D = 128
C = 1000
CORES = 8
NSH = N // CORES  # 125_000 rows per core
TILES = 980  # 980 * 128 = 125_440 padded rows per core
G = 49  # row-tiles per DMA chunk
CHUNKS = TILES // G  # 20
CPAD = 1024  # class dim padded (labels never reach 1000..1023)

# test.py can flip this before calling kernel() to capture a profile; the
# BassKernelResults of the last run is stored in LAST_RESULT either way.
TRACE = False
TRACE_KWARGS = {}
LAST_RESULT = None

_cached_nc = None


def _build_module():
    import concourse.mybir as mybir
    import concourse.tile as tile
    from concourse import bacc

    f16 = mybir.dt.float16
    f32 = mybir.dt.float32

    nc = bacc.Bacc(
        "TRN2",
        target_bir_lowering=False,
        debug=False,
        enable_asserts=False,
        num_devices=CORES,
    )
    emb_d = nc.dram_tensor("emb", [128, TILES * D], f16, kind="ExternalInput")
    lab_d = nc.dram_tensor("lab", [128, TILES], f32, kind="ExternalInput")
    iota_d = nc.dram_tensor("iota", [128, CPAD], f16, kind="ExternalInput")
    out_d = nc.dram_tensor("out", [128, CPAD], f32, kind="ExternalOutput")

    with tile.TileContext(nc) as tc:
        with (
            tc.tile_pool(name="consts", bufs=1) as consts,
            tc.tile_pool(name="ebuf", bufs=3) as ebuf,
            tc.tile_pool(name="obuf", bufs=4) as obuf,
            tc.tile_pool(name="psum", bufs=1, space="PSUM") as psum,
            tc.tile_pool(name="outb", bufs=1) as outb,
        ):
            lab_t = consts.tile([128, TILES], f32)
            iota_t = consts.tile([128, CPAD], f16)
            nc.sync.dma_start(out=lab_t[:], in_=lab_d[:])
            nc.sync.dma_start(out=iota_t[:], in_=iota_d[:])

            # Two PSUM banks accumulate [D=128, C=1024] fp32 across all tiles.
            psA = psum.tile([128, 512], f32)
            psB = psum.tile([128, 512], f32)

            for ch in range(CHUNKS):
                et = ebuf.tile([128, G * D], f16)
                nc.sync.dma_start(
                    out=et[:], in_=emb_d[:, ch * G * D : (ch + 1) * G * D]
                )
                for i in range(G):
                    t = ch * G + i
                    oh = obuf.tile([128, CPAD], f16)
                    nc.vector.tensor_scalar(
                        out=oh[:],
                        in0=iota_t[:],
                        scalar1=lab_t[:, t : t + 1],
                        scalar2=None,
                        op0=mybir.AluOpType.is_equal,
                    )
                    nc.tensor.matmul(
                        psA[:],
                        lhsT=et[:, i * D : (i + 1) * D],
                        rhs=oh[:, 0:512],
                        start=(t == 0),
                        stop=(t == TILES - 1),
                    )
                    nc.tensor.matmul(
                        psB[:],
                        lhsT=et[:, i * D : (i + 1) * D],
                        rhs=oh[:, 512:1024],
                        start=(t == 0),
                        stop=(t == TILES - 1),
                    )

            out_t = outb.tile([128, CPAD], f32)
            nc.vector.tensor_copy(out=out_t[:, 0:512], in_=psA[:])
            nc.vector.tensor_copy(out=out_t[:, 512:1024], in_=psB[:])
            nc.sync.dma_start(out=out_d[:], in_=out_t[:])

    nc.compile()
    return nc


def _prep_inputs(embeddings, labels):
    embeddings = np.asarray(embeddings)
    labels = np.asarray(labels).astype(np.int64)

    iota = np.ascontiguousarray(
        np.broadcast_to(np.arange(CPAD, dtype=np.float16), (128, CPAD))
    )
    in_maps = []
    for s in range(CORES):
        e = embeddings[s * NSH : (s + 1) * NSH]
        l = labels[s * NSH : (s + 1) * NSH]

        ep = np.zeros((TILES * 128, D), dtype=np.float16)
        ep[:NSH] = e.astype(np.float16)
        lp = np.full((TILES * 128,), -1.0, dtype=np.float32)
        lp[:NSH] = l.astype(np.float32)

        emb_t = np.ascontiguousarray(
            ep.reshape(TILES, 128, D).transpose(1, 0, 2)
        ).reshape(128, TILES * D)
        lab_t = np.ascontiguousarray(lp.reshape(TILES, 128).T)
        in_maps.append({"emb": emb_t, "lab": lab_t, "iota": iota})
    return in_maps


def kernel(embeddings, labels):
    global _cached_nc, LAST_RESULT
    from concourse.bass_utils import run_bass_kernel_spmd

    if _cached_nc is None:
        _cached_nc = _build_module()
    nc = _cached_nc

    in_maps = _prep_inputs(embeddings, labels)
    res = run_bass_kernel_spmd(
        nc,
        in_maps,
        core_ids=list(range(CORES)),
        trace=TRACE,
        **TRACE_KWARGS,
    )
    LAST_RESULT = res

    acc = np.zeros((128, CPAD), dtype=np.float64)
    for r in res.results:
        acc += r["out"].astype(np.float64)
    sums = acc.T[:C]  # [C, D]

    labels64 = np.asarray(labels).astype(np.int64)
    counts = np.bincount(labels64, minlength=C).astype(np.float64)

    means = sums / counts[:, None]
    mu = means.mean(axis=0)
    var = ((means - mu) ** 2).sum(axis=0) / (C - 1)
    return np.float32(-var.mean())


# revision 5
# speedup vs baseline: 1.2082x; 1.2082x over previous
# Trainium2 Bass kernel for nn_DiversityLoss (segment_reduce).
#
# reference:
#   sums   = segment_sum(embeddings, labels, C)        # [C, D]
#   counts = segment_sum(ones, labels, C)              # [C]
#   return -mean(var(sums / counts, axis=0, ddof=1))
#
# Strategy (data-parallel across 8 NeuronCores):
#   - Shard N=1M rows into 8 shards of 125k rows.
#   - On each core, compute the per-class partial sums of its shard with a
#     one-hot matmul on the Tensor engine:
#        for each 128-row tile t:
#           onehot[p, c] = (label[row p of t] == c)      (DVE is_equal vs iota)
#           psum[D, C]  += emb_tile[K=128rows, M=128D]^T @ onehot[K=128, N=C]
#     accumulated in PSUM (fp32) across all 980 tiles, then flushed to DRAM.
#   - Host: sum the 8 partial [D, C] outputs, counts via bincount on the
#     labels (0.2% of input bytes), then means/variance in float64.
#
# Layout prep on host (pure layout/dtype glue, no reduction math):
#   - embeddings cast fp32->fp16 and laid out [p, t, d]-contiguous per core so
#     each SBUF partition's DMA stream is fully contiguous.
#   - labels as fp16 in [p, t] layout (values 0..999 are exact in fp16).

import numpy as np

N = 1_000_000
D = 128
C = 1000
CORES = 8
NSH = N // CORES  # 125_000 rows per core
TILES = 980  # 980 * 128 = 125_440 padded rows per core
G = 49  # row-tiles per DMA chunk
CHUNKS = TILES // G  # 20
CPAD = 1024  # class dim padded (labels never reach 1000..1023)

# test.py can flip this before calling kernel() to capture a profile; the
# BassKernelResults of the last run is stored in LAST_RESULT either way.
TRACE = False
TRACE_KWARGS = {}
LAST_RESULT = None

_cached_nc = None


def _build_module():
    import concourse.mybir as mybir
    import concourse.tile as tile
    from concourse import bacc

    f16 = mybir.dt.float16
    f32 = mybir.dt.float32

    nc = bacc.Bacc(
        "TRN2",
        target_bir_lowering=False,
        debug=False,
        enable_asserts=False,
        num_devices=CORES,
    )
    emb_d = nc.dram_tensor("emb", [128, TILES * D], f16, kind="ExternalInput")
    lab_d = nc.dram_tensor("lab", [128, TILES], f32, kind="ExternalInput")
    iota_d = nc.dram_tensor("iota", [128, CPAD], f16, kind="ExternalInput")
    out_d = nc.dram_tensor("out", [128, CPAD], f32, kind="ExternalOutput")

    with tile.TileContext(nc) as tc:
        with (
            tc.tile_pool(name="consts", bufs=1) as consts,
            tc.tile_pool(name="ebuf", bufs=4) as ebuf,
            tc.tile_pool(name="obuf", bufs=6) as obuf,
            tc.tile_pool(name="psum", bufs=1, space="PSUM") as psum,
            tc.tile_pool(name="outb", bufs=1) as outb,
        ):
            lab_t = consts.tile([128, TILES], f32)
            iota_t = consts.tile([128, CPAD], f16)
            nc.sync.dma_start(out=lab_t[:], in_=lab_d[:])
            nc.sync.dma_start(out=iota_t[:], in_=iota_d[:])

            # Two PSUM banks accumulate [D=128, C=1024] fp32 across all tiles;
            # a third bank takes warmup matmuls that keep the PE HAM busy
            # while the first embedding chunk is still in flight.
            psA = psum.tile([128, 512], f32)
            psB = psum.tile([128, 512], f32)
            psW = psum.tile([128, 32], f32)
            for w in range(60):
                nc.tensor.matmul(
                    psW[:],
                    lhsT=iota_t[:, 0:128],
                    rhs=iota_t[:, 0:32],
                    start=True,
                    stop=True,
                    skip_group_check=True,
                )

            # First chunks are small so compute starts as soon as possible.
            splits = [0, 4, 12, 28]
            while splits[-1] < TILES:
                splits.append(min(splits[-1] + G, TILES))
            for ch in range(len(splits) - 1):
                t0, t1 = splits[ch], splits[ch + 1]
                et = ebuf.tile([128, G * D], f16)
                nc.sync.dma_start(
                    out=et[:, 0 : (t1 - t0) * D],
                    in_=emb_d[:, t0 * D : t1 * D],
                )
                for i in range(t1 - t0):
                    t = t0 + i
                    oh = obuf.tile([128, CPAD], f16)
                    nc.vector.tensor_scalar(
                        out=oh[:],
                        in0=iota_t[:],
                        scalar1=lab_t[:, t : t + 1],
                        scalar2=None,
                        op0=mybir.AluOpType.is_equal,
                    )
                    nc.tensor.matmul(
                        psA[:],
                        lhsT=et[:, i * D : (i + 1) * D],
                        rhs=oh[:, 0:512],
                        start=(t == 0),
                        stop=(t == TILES - 1),
                    )
                    nc.tensor.matmul(
                        psB[:],
                        lhsT=et[:, i * D : (i + 1) * D],
                        rhs=oh[:, 512:1024],
                        start=(t == 0),
                        stop=(t == TILES - 1),
                    )

            out_t = outb.tile([128, CPAD], f32)
            nc.vector.tensor_copy(out=out_t[:, 0:512], in_=psA[:])
            nc.vector.tensor_copy(out=out_t[:, 512:1024], in_=psB[:])
            nc.sync.dma_start(out=out_d[:], in_=out_t[:])

    nc.compile()
    return nc


def _prep_inputs(embeddings, labels):
    embeddings = np.asarray(embeddings)
    labels = np.asarray(labels).astype(np.int64)

    iota = np.ascontiguousarray(
        np.broadcast_to(np.arange(CPAD, dtype=np.float16), (128, CPAD))
    )
    in_maps = []
    for s in range(CORES):
        e = embeddings[s * NSH : (s + 1) * NSH]
        l = labels[s * NSH : (s + 1) * NSH]

        ep = np.zeros((TILES * 128, D), dtype=np.float16)
        ep[:NSH] = e.astype(np.float16)
        lp = np.full((TILES * 128,), -1.0, dtype=np.float32)
        lp[:NSH] = l.astype(np.float32)

        emb_t = np.ascontiguousarray(
            ep.reshape(TILES, 128, D).transpose(1, 0, 2)
        ).reshape(128, TILES * D)
        lab_t = np.ascontiguousarray(lp.reshape(TILES, 128).T)
        in_maps.append({"emb": emb_t, "lab": lab_t, "iota": iota})
    return in_maps


def kernel(embeddings, labels):
    global _cached_nc, LAST_RESULT
    from concourse.bass_utils import run_bass_kernel_spmd

    if _cached_nc is None:
        _cached_nc = _build_module()
    nc = _cached_nc

    in_maps = _prep_inputs(embeddings, labels)
    res = run_bass_kernel_spmd(
        nc,
        in_maps,
        core_ids=list(range(CORES)),
        trace=TRACE,
        **TRACE_KWARGS,
    )
    LAST_RESULT = res

    acc = np.zeros((128, CPAD), dtype=np.float64)
    for r in res.results:
        acc += r["out"].astype(np.float64)
    sums = acc.T[:C]  # [C, D]

    labels64 = np.asarray(labels).astype(np.int64)
    counts = np.bincount(labels64, minlength=C).astype(np.float64)

    means = sums / counts[:, None]
    mu = means.mean(axis=0)
    var = ((means - mu) ** 2).sum(axis=0) / (C - 1)
    return np.float32(-var.mean())


# revision 6
# speedup vs baseline: 1.2306x; 1.0185x over previous
# Trainium2 Bass kernel for nn_DiversityLoss (segment_reduce).
#
# reference:
#   sums   = segment_sum(embeddings, labels, C)        # [C, D]
#   counts = segment_sum(ones, labels, C)              # [C]
#   return -mean(var(sums / counts, axis=0, ddof=1))
#
# Strategy (data-parallel across 8 NeuronCores):
#   - Shard N=1M rows into 8 shards of 125k rows.
#   - On each core, compute the per-class partial sums of its shard with a
#     one-hot matmul on the Tensor engine:
#        for each 128-row tile t:
#           onehot[p, c] = (label[row p of t] == c)      (DVE is_equal vs iota)
#           psum[D, C]  += emb_tile[K=128rows, M=128D]^T @ onehot[K=128, N=C]
#     accumulated in PSUM (fp32) across all 980 tiles, then flushed to DRAM.
#   - Host: sum the 8 partial [D, C] outputs, counts via bincount on the
#     labels (0.2% of input bytes), then means/variance in float64.
#
# Layout prep on host (pure layout/dtype glue, no reduction math):
#   - embeddings cast fp32->fp16 and laid out [p, t, d]-contiguous per core so
#     each SBUF partition's DMA stream is fully contiguous.
#   - labels as fp16 in [p, t] layout (values 0..999 are exact in fp16).

import numpy as np

N = 1_000_000
D = 128
C = 1000
CORES = 8
NSH = N // CORES  # 125_000 rows per core
TILES = 977  # 977 * 128 = 125_056 padded rows per core
G = 49  # row-tiles per DMA chunk
CPAD_HALF = 500
CPAD = 1000  # exact class count; pad labels are -1 (never match)

# test.py can flip this before calling kernel() to capture a profile; the
# BassKernelResults of the last run is stored in LAST_RESULT either way.
TRACE = False
TRACE_KWARGS = {}
LAST_RESULT = None

_cached_nc = None


def _build_module():
    import concourse.mybir as mybir
    import concourse.tile as tile
    from concourse import bacc

    f16 = mybir.dt.float16
    f32 = mybir.dt.float32

    nc = bacc.Bacc(
        "TRN2",
        target_bir_lowering=False,
        debug=False,
        enable_asserts=False,
        num_devices=CORES,
    )
    emb_d = nc.dram_tensor("emb", [128, TILES * D], f16, kind="ExternalInput")
    lab_d = nc.dram_tensor("lab", [128, TILES], f32, kind="ExternalInput")
    iota_d = nc.dram_tensor("iota", [128, CPAD], f16, kind="ExternalInput")
    out_d = nc.dram_tensor("out", [128, CPAD], f32, kind="ExternalOutput")

    with tile.TileContext(nc) as tc:
        with (
            tc.tile_pool(name="consts", bufs=1) as consts,
            tc.tile_pool(name="ebuf", bufs=4) as ebuf,
            tc.tile_pool(name="obuf", bufs=6) as obuf,
            tc.tile_pool(name="psum", bufs=1, space="PSUM") as psum,
            tc.tile_pool(name="outb", bufs=1) as outb,
        ):
            lab_t = consts.tile([128, TILES], f32)
            iota_t = consts.tile([128, CPAD], f16)
            nc.sync.dma_start(out=iota_t[:], in_=iota_d[:])
            nc.sync.dma_start(out=lab_t[:, 0:28], in_=lab_d[:, 0:28])
            nc.sync.dma_start(out=lab_t[:, 28:TILES], in_=lab_d[:, 28:TILES])

            # Two PSUM banks accumulate [D=128, C=1024] fp32 across all tiles;
            # a third bank takes warmup matmuls that keep the PE HAM busy
            # while the first embedding chunk is still in flight.
            psA = psum.tile([128, CPAD_HALF], f32)
            psB = psum.tile([128, CPAD_HALF], f32)
            psW = psum.tile([128, 32], f32)
            for w in range(60):
                nc.tensor.matmul(
                    psW[:],
                    lhsT=iota_t[:, 0:128],
                    rhs=iota_t[:, 0:32],
                    start=True,
                    stop=True,
                    skip_group_check=True,
                )

            # First chunks are small so compute starts as soon as possible.
            splits = [0, 4, 12, 28]
            while splits[-1] < TILES:
                splits.append(min(splits[-1] + G, TILES))
            for ch in range(len(splits) - 1):
                t0, t1 = splits[ch], splits[ch + 1]
                et = ebuf.tile([128, G * D], f16)
                nc.sync.dma_start(
                    out=et[:, 0 : (t1 - t0) * D],
                    in_=emb_d[:, t0 * D : t1 * D],
                )
                for i in range(t1 - t0):
                    t = t0 + i
                    oh = obuf.tile([128, CPAD], f16)
                    nc.vector.tensor_scalar(
                        out=oh[:],
                        in0=iota_t[:],
                        scalar1=lab_t[:, t : t + 1],
                        scalar2=None,
                        op0=mybir.AluOpType.is_equal,
                    )
                    nc.tensor.matmul(
                        psA[:],
                        lhsT=et[:, i * D : (i + 1) * D],
                        rhs=oh[:, 0:CPAD_HALF],
                        start=(t == 0),
                        stop=(t == TILES - 1),
                    )
                    nc.tensor.matmul(
                        psB[:],
                        lhsT=et[:, i * D : (i + 1) * D],
                        rhs=oh[:, CPAD_HALF:CPAD],
                        start=(t == 0),
                        stop=(t == TILES - 1),
                    )

            out_t = outb.tile([128, CPAD], f32)
            nc.vector.tensor_copy(out=out_t[:, 0:CPAD_HALF], in_=psA[:])
            nc.vector.tensor_copy(out=out_t[:, CPAD_HALF:CPAD], in_=psB[:])
            nc.sync.dma_start(out=out_d[:], in_=out_t[:])

    nc.compile()
    return nc


def _prep_inputs(embeddings, labels):
    embeddings = np.asarray(embeddings)
    labels = np.asarray(labels).astype(np.int64)

    iota = np.ascontiguousarray(
        np.broadcast_to(np.arange(CPAD, dtype=np.float16), (128, CPAD))
    )
    in_maps = []
    for s in range(CORES):
        e = embeddings[s * NSH : (s + 1) * NSH]
        l = labels[s * NSH : (s + 1) * NSH]

        ep = np.zeros((TILES * 128, D), dtype=np.float16)
        ep[:NSH] = e.astype(np.float16)
        lp = np.full((TILES * 128,), -1.0, dtype=np.float32)
        lp[:NSH] = l.astype(np.float32)

        emb_t = np.ascontiguousarray(
            ep.reshape(TILES, 128, D).transpose(1, 0, 2)
        ).reshape(128, TILES * D)
        lab_t = np.ascontiguousarray(lp.reshape(TILES, 128).T)
        in_maps.append({"emb": emb_t, "lab": lab_t, "iota": iota})
    return in_maps


def kernel(embeddings, labels):
    global _cached_nc, LAST_RESULT
    from concourse.bass_utils import run_bass_kernel_spmd

    if _cached_nc is None:
        _cached_nc = _build_module()
    nc = _cached_nc

    in_maps = _prep_inputs(embeddings, labels)
    res = run_bass_kernel_spmd(
        nc,
        in_maps,
        core_ids=list(range(CORES)),
        trace=TRACE,
        **TRACE_KWARGS,
    )
    LAST_RESULT = res

    acc = np.zeros((128, CPAD), dtype=np.float64)
    for r in res.results:
        acc += r["out"].astype(np.float64)
    sums = acc.T[:C]  # [C, D]

    labels64 = np.asarray(labels).astype(np.int64)
    counts = np.bincount(labels64, minlength=C).astype(np.float64)

    means = sums / counts[:, None]
    mu = means.mean(axis=0)
    var = ((means - mu) ** 2).sum(axis=0) / (C - 1)
    return np.float32(-var.mean())
